# revision 8
# baseline (speedup 1.0000x reference)
"""Bass/Tile Trainium2 kernel for masked dot-product attention.

Problem: Q,K,V [2,16,2048,64] f32, attn_mask [2,1,2048,2048] bool (True = masked).
out = softmax(QK^T/8 masked) @ V, computed on 8 NeuronCores, batch*head sharded
(4 heads per core, each core's heads all in one batch so it needs one mask).

Device-side layout ("layout C" — transposed scores, row-tiled QK pairs):
  k-tiles t and t+8 form pair p=t; K^T for tile p sits in SBUF partitions
  0-63, tile p+8 in partitions 64-127 (Q^T is duplicated into both halves).
  Per (pair p, q-chunk qc of 512):
    st2[128, 1024] f32 PSUM (2 banks):
      st2[:, 0:512]    = K_p  @ Q^T chunk   (TensorE tile_position (0,0))
      st2[:, 512:1024] = K_p8 @ Q^T chunk   (TensorE tile_position (64,0))
      -> the two 64-contraction matmuls run CONCURRENTLY in the PE array.
    et2[128, 1024] bf16 = exp(st2 / 8)      (one ScalarE call, N=1024)
    et2 *= keep2[p, qc]                     (one VectorE bf16 mul; keep = ~mask)
    ots[qc][65, 512] += Vp_p^T  @ et2[:, 0:512]     (TensorE, accum in PSUM;
    ots[qc][65, 512] += Vp_p8^T @ et2[:, 512:1024]   row 64 = ones -> denom)
  AV is lagged by `avlag` groups so the in-order PE stream never stalls on
  the exp/mask chain ahead of the next QK pair. ScalarE (exp) is the
  bottleneck engine; everything else hides under it.
  Tail per q-chunk: DVE copy ev<-ots (frees PSUM), reciprocal_approx_fast
  on the denominator row, gpsimd partition-broadcast, bf16 multiply, DMA
  out in [d, q] bf16 (host transposes and casts to f32).
"""

import numpy as np
import ml_dtypes

B, H, S, DK = 2, 16, 2048, 64
NCORES = 8
HPC = (B * H) // NCORES  # heads per core = 4
KT = S // 128            # 16 k-tiles
NP = KT // 2             # 8 pairs
NG = NP * 4              # 32 groups (pair, q-chunk) per head
VPW = 72                 # v-block stride (64 v + 1 ones + pad to 16B align)
BF16 = ml_dtypes.bfloat16

_CACHE = {}


def _build(hpc=HPC, compile=True, reps=1,
           stages=("qk", "exp", "mask", "av", "tail"), avlag=5,
           iobufs=2, ebufs=8, stbufs=2, **_ignored):
    import contextlib
    import concourse.bass as bass
    import concourse.mybir as mybir
    import concourse.tile as tile
    from concourse import bacc
    HPC = hpc
    stages = set(stages)
    f32 = mybir.dt.float32
    bf16 = mybir.dt.bfloat16
    Exp = mybir.ActivationFunctionType.Exp

    nc = bacc.Bacc("TRN2", target_bir_lowering=False, debug=False,
                   num_devices=NCORES)

    qt_d = nc.dram_tensor("qt", [HPC, 128, S], bf16, kind="ExternalInput").ap()
    kt_d = nc.dram_tensor("kt", [HPC, 128, NP * 128], bf16,
                          kind="ExternalInput").ap()
    vp_d = nc.dram_tensor("vp", [HPC, 128, KT * VPW], bf16,
                          kind="ExternalInput").ap()
    mk_d = nc.dram_tensor("maskt", [128, NG * 1024], bf16,
                          kind="ExternalInput").ap()
    out_d = nc.dram_tensor("out", [HPC, DK, S], bf16, kind="ExternalOutput").ap()

    with tile.TileContext(nc) as tc:
        with (
            tc.tile_pool(name="const", bufs=1) as const,
            tc.tile_pool(name="io", bufs=iobufs) as io,
            tc.tile_pool(name="epool", bufs=ebufs) as epool,
            tc.tile_pool(name="fin", bufs=4) as fin,
            tc.tile_pool(name="ps_s", bufs=stbufs, space="PSUM") as ps_s,
            tc.tile_pool(name="ps_o", bufs=4, space="PSUM") as ps_o,
        ):
            mask_sb = const.tile([128, NG * 1024], bf16)
            mask_v = mask_sb.rearrange("p (g q) -> p g q", g=NG)
            mk_v = mk_d.rearrange("p (g q) -> p g q", g=NG)

            loop_ctx = (tc.For_i(0, reps, 1) if reps > 1
                        else contextlib.nullcontext())

            state = {}

            NTAIL = 10

            def emit_tail_piece(h, step):
                # row 0 of ots is the denominator (ones column first in vp).
                # steps 0-3: evacuate ots -> one ev tile (frees PSUM banks)
                # step 4: reciprocal of the denominator row (all 2048 q)
                # step 5: gpsimd partition-broadcast to 65 rows
                # steps 6/8: bf16 normalize muls; steps 7/9: output DMA
                ots, ost, tl = state[h]
                if step < 4:
                    qc = step
                    if tl.get("ev") is None:
                        tl["ev"] = fin.tile([65, S], f32, tag="ev", bufs=2,
                                            name=f"ev_h{h}")
                    nc.vector.tensor_copy(
                        tl["ev"][:, qc * 512:(qc + 1) * 512], ots[qc])
                elif step == 4:
                    tl["rc"] = fin.tile([1, S], f32, tag="rc", bufs=2,
                                        name=f"rc_h{h}")
                    nc.vector.reciprocal_approx_fast(tl["rc"],
                                                     tl["ev"][0:1, :])
                elif step == 5:
                    tl["bc"] = fin.tile([65, S], f32, tag="bc", bufs=2,
                                        name=f"bc_h{h}")
                    nc.gpsimd.partition_broadcast(tl["bc"], tl["rc"])
                elif step in (6, 8):
                    sl = slice(0, 1024) if step == 6 else slice(1024, 2048)
                    nc.vector.tensor_mul(ost[:, sl], tl["ev"][:, sl],
                                         tl["bc"][:, sl])
                else:
                    sl = slice(0, 1024) if step == 7 else slice(1024, 2048)
                    nc.gpsimd.dma_start(out=out_d[h][:, sl],
                                        in_=ost[1:65, sl])

            def emit_av(h, g):
                ots, _, _ = state[h]
                p, qc = divmod(g, 4)
                et = state[(h, "et")][g % ebufs]
                vp_v = state[(h, "vp")]
                nc.tensor.matmul(ots[qc], vp_v[:, p, :65], et[:, 0:512],
                                 start=(p == 0), stop=False)
                nc.tensor.matmul(ots[qc], vp_v[:, p + NP, :65],
                                 et[:, 512:1024],
                                 start=False, stop=(p == NP - 1))

            with loop_ctx:
                for h in range(HPC + 1):
                    if h < HPC:
                        qt_sb = io.tile([128, S], bf16, tag="qt")
                        nc.sync.dma_start(out=qt_sb[:, :1024],
                                          in_=qt_d[h][:, :1024])
                        kt_sb = io.tile([128, NP * 128], bf16, tag="kt")
                        nc.sync.dma_start(out=kt_sb, in_=kt_d[h])
                        nc.sync.dma_start(out=qt_sb[:, 1024:],
                                          in_=qt_d[h][:, 1024:])
                        vp_sb = io.tile([128, KT * VPW], bf16, tag="vp")
                        nc.sync.dma_start(out=vp_sb, in_=vp_d[h])
                        vp_v = vp_sb.rearrange("p (k c) -> p k c", k=KT)
                        if h == 0:
                            for i in range(8):
                                nc.sync.dma_start(
                                    out=mask_sb[:, i * 4096:(i + 1) * 4096],
                                    in_=mk_d[:, i * 4096:(i + 1) * 4096])
                        ots = []
                        for qc in range(4):
                            ot = ps_o.tile([65, 512], f32, tag="ot",
                                           name=f"ot_h{h}_q{qc}")
                            ots.append(ot)
                        ost = io.tile([65, S], bf16, tag="ost",
                                      name=f"ost_h{h}")
                        state[h] = (ots, ost, {})
                        state[(h, "et")] = [None] * ebufs
                        state[(h, "vp")] = vp_v

                    if h == HPC:
                        # flush the last head's tail
                        if "tail" in stages:
                            state[HPC] = state[HPC - 1]
                            state[(HPC, "et")] = state[(HPC - 1, "et")]
                            state[(HPC, "vp")] = state[(HPC - 1, "vp")]
                            for step in range(NTAIL):
                                emit_tail_piece(HPC - 1, step)
                        break

                    for g in range(NG):
                        p, qc = divmod(g, 4)
                        st2 = ps_s.tile([128, 1024], f32, tag="st",
                                        name=f"st_h{h}_g{g}")
                        if "qk" in stages:
                            nc.tensor.matmul(
                                st2[:, 0:512],
                                kt_sb[0:64, p * 128:(p + 1) * 128],
                                qt_sb[0:64, qc * 512:(qc + 1) * 512],
                                start=True, stop=True, tile_position=(0, 0))
                            nc.tensor.matmul(
                                st2[:, 512:1024],
                                kt_sb[64:128, p * 128:(p + 1) * 128],
                                qt_sb[64:128, qc * 512:(qc + 1) * 512],
                                start=True, stop=True, tile_position=(64, 0))
                        et2 = epool.tile([128, 1024], bf16, tag="et",
                                         name=f"et_h{h}_g{g}")
                        state[(h, "et")][g % ebufs] = et2
                        if "exp" in stages:
                            nc.scalar.activation(et2, st2, Exp,
                                                 scale=1.0 / np.sqrt(DK))
                        if "mask" in stages:
                            nc.vector.tensor_mul(et2, et2, mask_v[:, g, :])
                        if "av" in stages:
                            if g >= avlag:
                                emit_av(h, g - avlag)
                            if g == NG - 1:
                                for gg in range(NG - avlag, NG):
                                    emit_av(h, gg)
                        if (h > 0 and "tail" in stages and g % 2 == 0
                                and g // 2 < NTAIL):
                            emit_tail_piece(h - 1, g // 2)

    if compile:
        nc.compile()
    return nc


def _get_nc():
    if "nc" not in _CACHE:
        _CACHE["nc"] = _build()
    return _CACHE["nc"]


def _shard(Q, K, V, attn_mask):
    """Host-side marshalling: shard/transposes per core."""
    Q = np.asarray(Q, np.float32)
    K = np.asarray(K, np.float32)
    V = np.asarray(V, np.float32)
    attn_mask = np.asarray(attn_mask, bool)

    # keep2[b][128, g=(p,qc), 1024] = [keepT(tile p) | keepT(tile p+8)] for
    # q columns qc*512:(qc+1)*512, where keepT[b, kp, t, q] = ~mask[b, q, k]
    keep = (~attn_mask[:, 0]).astype(BF16)                   # [B, q, k]
    mkT = keep.transpose(0, 2, 1)                            # [B, k, q]
    mkT = mkT.reshape(B, KT, 128, S)                         # [B, t, kp, q]
    # -> [B, 128, p, qc, half, 512]
    m2 = np.empty((B, 128, NP, 4, 2, 512), BF16)
    for p in range(NP):
        for qc in range(4):
            m2[:, :, p, qc, 0, :] = mkT[:, p, :, qc * 512:(qc + 1) * 512]
            m2[:, :, p, qc, 1, :] = mkT[:, p + NP, :, qc * 512:(qc + 1) * 512]
    m2 = np.ascontiguousarray(m2).reshape(B, 128, NG * 1024)

    in_maps = []
    for c in range(NCORES):
        b = c // (NCORES // B)
        h0 = (c % (NCORES // B)) * HPC
        QT = np.ascontiguousarray(
            Q[b, h0:h0 + HPC].transpose(0, 2, 1)).astype(BF16)  # [HPC, DK, S]
        QT2 = np.concatenate([QT, QT], axis=1)                  # [HPC, 128, S]
        KTt = np.ascontiguousarray(
            K[b, h0:h0 + HPC].transpose(0, 2, 1)).astype(BF16)  # [HPC, DK, S]
        # kt[h, 0:64, p*128:...] = tile p; kt[h, 64:128, ...] = tile p+8
        kt2 = np.empty((HPC, 128, NP * 128), BF16)
        kt2[:, 0:64, :] = KTt[:, :, :NP * 128]
        kt2[:, 64:128, :] = KTt[:, :, NP * 128:]
        vp = np.zeros((HPC, 128, KT, VPW), BF16)
        vp[:, :, :, 0] = 1.0
        vp[:, :, :, 1:DK + 1] = V[b, h0:h0 + HPC].astype(BF16).reshape(
            HPC, KT, 128, DK).transpose(0, 2, 1, 3)
        in_maps.append({
            "qt": QT2,
            "kt": kt2,
            "vp": np.ascontiguousarray(vp).reshape(HPC, 128, KT * VPW),
            "maskt": m2[b],
        })
    return in_maps


def kernel(Q, K, V, attn_mask):
    from concourse.bass_utils import run_bass_kernel_spmd

    nc = _get_nc()
    in_maps = _shard(Q, K, V, attn_mask)
    res = run_bass_kernel_spmd(nc, in_maps, list(range(NCORES)))
    out = np.empty((B, H, S, DK), np.float32)
    for c in range(NCORES):
        b = c // (NCORES // B)
        h0 = (c % (NCORES // B)) * HPC
        out[b, h0:h0 + HPC] = res.results[c]["out"].astype(
            np.float32).transpose(0, 2, 1)
    return out


# revision 9
# speedup vs baseline: 1.1842x; 1.1842x over previous
"""Bass/Tile Trainium2 kernel for masked dot-product attention.

Problem: Q,K,V [2,16,2048,64] f32, attn_mask [2,1,2048,2048] bool (True = masked).
out = softmax(QK^T/8 masked) @ V, computed on 8 NeuronCores, batch*head sharded
(4 heads per core, each core's heads all in one batch so it needs one mask).

Device-side layout ("layout C" — transposed scores, row-tiled QK pairs):
  k-tiles t and t+8 form pair p=t; K^T for tile p sits in SBUF partitions
  0-63, tile p+8 in partitions 64-127 (Q^T is duplicated into both halves).
  Per (pair p, q-chunk qc of 512):
    st2[128, 1024] f32 PSUM (2 banks):
      st2[:, 0:512]    = K_p  @ Q^T chunk   (TensorE tile_position (0,0))
      st2[:, 512:1024] = K_p8 @ Q^T chunk   (TensorE tile_position (64,0))
      -> the two 64-contraction matmuls run CONCURRENTLY in the PE array.
    et2[128, 1024] bf16 = exp(st2 / 8)      (one ScalarE call, N=1024)
    et2 *= keep2[p, qc]                     (one VectorE bf16 mul; keep = ~mask)
    ots[qc][65, 512] += Vp_p^T  @ et2[:, 0:512]     (TensorE, accum in PSUM;
    ots[qc][65, 512] += Vp_p8^T @ et2[:, 512:1024]   row 64 = ones -> denom)
  AV is lagged by `avlag` groups so the in-order PE stream never stalls on
  the exp/mask chain ahead of the next QK pair. ScalarE (exp) is the
  bottleneck engine; everything else hides under it.
  Tail per q-chunk: DVE copy ev<-ots (frees PSUM), reciprocal_approx_fast
  on the denominator row, gpsimd partition-broadcast, bf16 multiply, DMA
  out in [d, q] bf16 (host transposes and casts to f32).
"""

import numpy as np
import ml_dtypes

B, H, S, DK = 2, 16, 2048, 64
NCORES = 8
HPC = (B * H) // NCORES  # heads per core = 4
KT = S // 128            # 16 k-tiles
NP = KT // 2             # 8 pairs
NG = NP * 4              # 32 groups (pair, q-chunk) per head
VPW = 72                 # v-block stride (64 v + 1 ones + pad to 16B align)
BF16 = ml_dtypes.bfloat16

_CACHE = {}


def _build(hpc=HPC, compile=True, reps=1,
           stages=("qk", "exp", "mask", "av", "tail"), avlag=4,
           iobufs=2, ebufs=8, stbufs=2, tailspread=1, **_ignored):
    import contextlib
    import concourse.bass as bass
    import concourse.mybir as mybir
    import concourse.tile as tile
    from concourse import bacc
    HPC = hpc
    stages = set(stages)
    f32 = mybir.dt.float32
    bf16 = mybir.dt.bfloat16
    Exp = mybir.ActivationFunctionType.Exp

    nc = bacc.Bacc("TRN2", target_bir_lowering=False, debug=False,
                   num_devices=NCORES)

    qt_d = nc.dram_tensor("qt", [HPC, 128, S], bf16, kind="ExternalInput").ap()
    kt_d = nc.dram_tensor("kt", [HPC, 128, NP * 128], bf16,
                          kind="ExternalInput").ap()
    vp_d = nc.dram_tensor("vp", [HPC, 128, KT * VPW], bf16,
                          kind="ExternalInput").ap()
    mk_d = nc.dram_tensor("maskt", [128, NG * 1024], bf16,
                          kind="ExternalInput").ap()
    out_d = nc.dram_tensor("out", [HPC, DK, S], bf16, kind="ExternalOutput").ap()

    with tile.TileContext(nc) as tc:
        with (
            tc.tile_pool(name="const", bufs=1) as const,
            tc.tile_pool(name="io", bufs=iobufs) as io,
            tc.tile_pool(name="epool", bufs=ebufs) as epool,
            tc.tile_pool(name="fin", bufs=4) as fin,
            tc.tile_pool(name="ps_s", bufs=stbufs, space="PSUM") as ps_s,
            tc.tile_pool(name="ps_o", bufs=4, space="PSUM") as ps_o,
        ):
            mask_sb = const.tile([128, NG * 1024], bf16)
            mask_v = mask_sb.rearrange("p (g q) -> p g q", g=NG)
            mk_v = mk_d.rearrange("p (g q) -> p g q", g=NG)

            loop_ctx = (tc.For_i(0, reps, 1) if reps > 1
                        else contextlib.nullcontext())

            state = {}

            NTAIL = 12

            def emit_tail_piece(h, step):
                # row 0 of ots is the denominator (ones column first in vp).
                # step 0-3: evacuate ots -> ev (frees PSUM for next head)
                # step 4-7: reciprocal of denominator row + gpsimd broadcast
                # step 8-11: final multiply + store chunk
                ots, ost, tl = state[h]
                qc = step % 4
                sl = slice(qc * 512, (qc + 1) * 512)
                if step < 4:
                    ev = fin.tile([65, 512], f32, tag="ev", bufs=8,
                                  name=f"ev_h{h}_q{qc}")
                    nc.vector.tensor_copy(ev, ots[qc])
                    tl[("ev", qc)] = ev
                elif step < 8:
                    rc1 = fin.tile([1, 512], f32, tag="rc1", bufs=8,
                                   name=f"rc1_h{h}_q{qc}")
                    nc.vector.reciprocal_approx_fast(rc1, tl[("ev", qc)][0:1, :])
                    bc = fin.tile([65, 512], f32, tag="bc", bufs=4,
                                  name=f"bc_h{h}_q{qc}")
                    nc.gpsimd.partition_broadcast(bc, rc1)
                    tl[("bc", qc)] = bc
                else:
                    nc.vector.tensor_mul(ost[:, sl], tl[("ev", qc)],
                                         tl[("bc", qc)])
                    nc.gpsimd.dma_start(out=out_d[h][:, sl],
                                        in_=ost[1:65, sl])

            def emit_av(h, g):
                ots, _, _ = state[h]
                p, qc = divmod(g, 4)
                et = state[(h, "et")][g % ebufs]
                vp_v = state[(h, "vp")]
                nc.tensor.matmul(ots[qc], vp_v[:, p, :65], et[:, 0:512],
                                 start=(p == 0), stop=False)
                nc.tensor.matmul(ots[qc], vp_v[:, p + NP, :65],
                                 et[:, 512:1024],
                                 start=False, stop=(p == NP - 1))

            with loop_ctx:
                for h in range(HPC + 1):
                    if h < HPC:
                        qt_sb = io.tile([128, S], bf16, tag="qt")
                        nc.sync.dma_start(out=qt_sb[:, :1024],
                                          in_=qt_d[h][:, :1024])
                        kt_sb = io.tile([128, NP * 128], bf16, tag="kt")
                        nc.sync.dma_start(out=kt_sb, in_=kt_d[h])
                        nc.sync.dma_start(out=qt_sb[:, 1024:],
                                          in_=qt_d[h][:, 1024:])
                        vp_sb = io.tile([128, KT * VPW], bf16, tag="vp")
                        nc.sync.dma_start(out=vp_sb, in_=vp_d[h])
                        vp_v = vp_sb.rearrange("p (k c) -> p k c", k=KT)
                        if h == 0:
                            for i in range(8):
                                nc.sync.dma_start(
                                    out=mask_sb[:, i * 4096:(i + 1) * 4096],
                                    in_=mk_d[:, i * 4096:(i + 1) * 4096])
                        ots = []
                        for qc in range(4):
                            ot = ps_o.tile([65, 512], f32, tag="ot",
                                           name=f"ot_h{h}_q{qc}")
                            ots.append(ot)
                        ost = io.tile([65, S], bf16, tag="ost",
                                      name=f"ost_h{h}")
                        state[h] = (ots, ost, {})
                        state[(h, "et")] = [None] * ebufs
                        state[(h, "vp")] = vp_v

                    if h == HPC:
                        # flush the last head's tail
                        if "tail" in stages:
                            state[HPC] = state[HPC - 1]
                            state[(HPC, "et")] = state[(HPC - 1, "et")]
                            state[(HPC, "vp")] = state[(HPC - 1, "vp")]
                            for step in range(NTAIL):
                                emit_tail_piece(HPC - 1, step)
                        break

                    for g in range(NG):
                        p, qc = divmod(g, 4)
                        st2 = ps_s.tile([128, 1024], f32, tag="st",
                                        name=f"st_h{h}_g{g}")
                        if "qk" in stages:
                            nc.tensor.matmul(
                                st2[:, 0:512],
                                kt_sb[0:64, p * 128:(p + 1) * 128],
                                qt_sb[0:64, qc * 512:(qc + 1) * 512],
                                start=True, stop=True, tile_position=(0, 0))
                            nc.tensor.matmul(
                                st2[:, 512:1024],
                                kt_sb[64:128, p * 128:(p + 1) * 128],
                                qt_sb[64:128, qc * 512:(qc + 1) * 512],
                                start=True, stop=True, tile_position=(64, 0))
                        et2 = epool.tile([128, 1024], bf16, tag="et",
                                         name=f"et_h{h}_g{g}")
                        state[(h, "et")][g % ebufs] = et2
                        if "exp" in stages:
                            nc.scalar.activation(et2, st2, Exp,
                                                 scale=1.0 / np.sqrt(DK))
                        if "mask" in stages:
                            nc.vector.tensor_mul(et2, et2, mask_v[:, g, :])
                        if "av" in stages:
                            if g >= avlag:
                                emit_av(h, g - avlag)
                            if g == NG - 1:
                                for gg in range(NG - avlag, NG):
                                    emit_av(h, gg)
                        if (h > 0 and "tail" in stages
                                and g % tailspread == 0
                                and g // tailspread < NTAIL):
                            emit_tail_piece(h - 1, g // tailspread)

    if compile:
        nc.compile()
    return nc


def _get_nc():
    if "nc" not in _CACHE:
        _CACHE["nc"] = _build()
    return _CACHE["nc"]


def _shard(Q, K, V, attn_mask):
    """Host-side marshalling: shard/transposes per core."""
    Q = np.asarray(Q, np.float32)
    K = np.asarray(K, np.float32)
    V = np.asarray(V, np.float32)
    attn_mask = np.asarray(attn_mask, bool)

    # keep2[b][128, g=(p,qc), 1024] = [keepT(tile p) | keepT(tile p+8)] for
    # q columns qc*512:(qc+1)*512, where keepT[b, kp, t, q] = ~mask[b, q, k]
    keep = (~attn_mask[:, 0]).astype(BF16)                   # [B, q, k]
    mkT = keep.transpose(0, 2, 1)                            # [B, k, q]
    mkT = mkT.reshape(B, KT, 128, S)                         # [B, t, kp, q]
    # -> [B, 128, p, qc, half, 512]
    m2 = np.empty((B, 128, NP, 4, 2, 512), BF16)
    for p in range(NP):
        for qc in range(4):
            m2[:, :, p, qc, 0, :] = mkT[:, p, :, qc * 512:(qc + 1) * 512]
            m2[:, :, p, qc, 1, :] = mkT[:, p + NP, :, qc * 512:(qc + 1) * 512]
    m2 = np.ascontiguousarray(m2).reshape(B, 128, NG * 1024)

    in_maps = []
    for c in range(NCORES):
        b = c // (NCORES // B)
        h0 = (c % (NCORES // B)) * HPC
        QT = np.ascontiguousarray(
            Q[b, h0:h0 + HPC].transpose(0, 2, 1)).astype(BF16)  # [HPC, DK, S]
        QT2 = np.concatenate([QT, QT], axis=1)                  # [HPC, 128, S]
        KTt = np.ascontiguousarray(
            K[b, h0:h0 + HPC].transpose(0, 2, 1)).astype(BF16)  # [HPC, DK, S]
        # kt[h, 0:64, p*128:...] = tile p; kt[h, 64:128, ...] = tile p+8
        kt2 = np.empty((HPC, 128, NP * 128), BF16)
        kt2[:, 0:64, :] = KTt[:, :, :NP * 128]
        kt2[:, 64:128, :] = KTt[:, :, NP * 128:]
        vp = np.zeros((HPC, 128, KT, VPW), BF16)
        vp[:, :, :, 0] = 1.0
        vp[:, :, :, 1:DK + 1] = V[b, h0:h0 + HPC].astype(BF16).reshape(
            HPC, KT, 128, DK).transpose(0, 2, 1, 3)
        in_maps.append({
            "qt": QT2,
            "kt": kt2,
            "vp": np.ascontiguousarray(vp).reshape(HPC, 128, KT * VPW),
            "maskt": m2[b],
        })
    return in_maps


def kernel(Q, K, V, attn_mask):
    from concourse.bass_utils import run_bass_kernel_spmd

    nc = _get_nc()
    in_maps = _shard(Q, K, V, attn_mask)
    res = run_bass_kernel_spmd(nc, in_maps, list(range(NCORES)))
    out = np.empty((B, H, S, DK), np.float32)
    for c in range(NCORES):
        b = c // (NCORES // B)
        h0 = (c % (NCORES // B)) * HPC
        out[b, h0:h0 + HPC] = res.results[c]["out"].astype(
            np.float32).transpose(0, 2, 1)
    return out


# revision 14
# speedup vs baseline: 1.2792x; 1.0803x over previous
"""Bass/Tile Trainium2 kernel for masked dot-product attention.

Problem: Q,K,V [2,16,2048,64] f32, attn_mask [2,1,2048,2048] bool (True = masked).
out = softmax(QK^T/8 masked) @ V, computed on 8 NeuronCores, batch*head sharded
(4 heads per core, each core's heads all in one batch so it needs one mask).

Device-side layout ("layout C" — transposed scores, row-tiled QK pairs):
  k-tiles t and t+8 form pair p=t; K^T for tile p sits in SBUF partitions
  0-63, tile p+8 in partitions 64-127 (Q^T is duplicated into both halves).
  Per (pair p, q-chunk qc of 512):
    st2[128, 1024] f32 PSUM (2 banks):
      st2[:, 0:512]    = K_p  @ Q^T chunk   (TensorE tile_position (0,0))
      st2[:, 512:1024] = K_p8 @ Q^T chunk   (TensorE tile_position (64,0))
      -> the two 64-contraction matmuls run CONCURRENTLY in the PE array.
    et2[128, 1024] bf16 = exp(st2 / 8)      (one ScalarE call, N=1024)
    et2 *= keep2[p, qc]                     (one VectorE bf16 mul; keep = ~mask)
    ots[qc][65, 512] += Vp_p^T  @ et2[:, 0:512]     (TensorE, accum in PSUM;
    ots[qc][65, 512] += Vp_p8^T @ et2[:, 512:1024]   row 64 = ones -> denom)
  AV is lagged by `avlag` groups so the in-order PE stream never stalls on
  the exp/mask chain ahead of the next QK pair. ScalarE (exp) is the
  bottleneck engine; everything else hides under it.
  Tail per q-chunk: DVE copy ev<-ots (frees PSUM), reciprocal_approx_fast
  on the denominator row, gpsimd partition-broadcast, bf16 multiply, DMA
  out in [d, q] bf16 (host transposes and casts to f32).
"""

import numpy as np
import ml_dtypes

B, H, S, DK = 2, 16, 2048, 64
NCORES = 8
HPC = (B * H) // NCORES  # heads per core = 4
KT = S // 128            # 16 k-tiles
NP = KT // 2             # 8 pairs
NG = NP * 4              # 32 groups (pair, q-chunk) per head
VPW = 72                 # v-block stride (64 v + 1 ones + pad to 16B align)
BF16 = ml_dtypes.bfloat16

_CACHE = {}


def _build(hpc=HPC, compile=True, reps=1,
           stages=("qk", "exp", "mask", "av", "tail"), avlag=4,
           iobufs=2, ebufs=8, stbufs=2, tailspread=1, tailsched=None,
           bcdma=False, outsync=False, tailmode="full", **_ignored):
    import contextlib
    import concourse.bass as bass
    import concourse.mybir as mybir
    import concourse.tile as tile
    from concourse import bacc
    HPC = hpc
    stages = set(stages)
    f32 = mybir.dt.float32
    bf16 = mybir.dt.bfloat16
    Exp = mybir.ActivationFunctionType.Exp

    nc = bacc.Bacc("TRN2", target_bir_lowering=False, debug=False,
                   num_devices=NCORES)

    qt_d = nc.dram_tensor("qt", [HPC, 128, S], bf16, kind="ExternalInput").ap()
    kt_d = nc.dram_tensor("kt", [HPC, 128, NP * 128], bf16,
                          kind="ExternalInput").ap()
    vp_d = nc.dram_tensor("vp", [HPC, 128, KT * VPW], bf16,
                          kind="ExternalInput").ap()
    mk_d = nc.dram_tensor("maskt", [128, NG * 1024], bf16,
                          kind="ExternalInput").ap()
    out_d = nc.dram_tensor("out", [HPC, DK, S], bf16, kind="ExternalOutput").ap()
    scr_d = nc.dram_tensor("scr", [8, 512], mybir.dt.float32, kind="Internal").ap()

    with tile.TileContext(nc) as tc:
        with (
            tc.tile_pool(name="const", bufs=1) as const,
            tc.tile_pool(name="io", bufs=iobufs) as io,
            tc.tile_pool(name="epool", bufs=ebufs) as epool,
            tc.tile_pool(name="fin", bufs=4) as fin,
            tc.tile_pool(name="ps_s", bufs=stbufs, space="PSUM") as ps_s,
            tc.tile_pool(name="ps_o", bufs=4, space="PSUM") as ps_o,
        ):
            mask_sb = const.tile([128, NG * 1024], bf16)
            mask_v = mask_sb.rearrange("p (g q) -> p g q", g=NG)
            mk_v = mk_d.rearrange("p (g q) -> p g q", g=NG)

            loop_ctx = (tc.For_i(0, reps, 1) if reps > 1
                        else contextlib.nullcontext())

            state = {}

            NTAIL = 12

            def emit_tail_piece(h, step):
                # row 0 of ots is the denominator (ones column first in vp).
                # step 0-3: evacuate ots -> ev (frees PSUM for next head)
                # step 4-7: reciprocal of denominator row + gpsimd broadcast
                # step 8-11: final multiply + store chunk
                ots, ost, tl = state[h]
                qc = step % 4
                sl = slice(qc * 512, (qc + 1) * 512)
                if step < 4:
                    ev = fin.tile([65, 512], f32, tag="ev", bufs=8,
                                  name=f"ev_h{h}_q{qc}")
                    nc.vector.tensor_copy(ev, ots[qc])
                    tl[("ev", qc)] = ev
                elif step < 8:
                    if tailmode == "evonly":
                        return
                    rc1 = fin.tile([1, 512], f32, tag="rc1", bufs=8,
                                   name=f"rc1_h{h}_q{qc}")
                    nc.vector.reciprocal_approx_fast(rc1, tl[("ev", qc)][0:1, :])
                    if tailmode == "bf16bc":
                        rcb = fin.tile([1, 512], bf16, tag="rcb", bufs=8,
                                       name=f"rcb_h{h}_q{qc}")
                        nc.vector.tensor_copy(rcb, rc1)
                        bc = fin.tile([65, 512], bf16, tag="bc", bufs=4,
                                      name=f"bc_h{h}_q{qc}")
                        nc.gpsimd.partition_broadcast(bc, rcb)
                    elif tailmode == "drambc":
                        sslot = scr_d[(h % 2) * 4 + qc]
                        nc.sync.dma_start(out=sslot, in_=rc1)
                        bc = fin.tile([65, 512], f32, tag="bc", bufs=4,
                                      name=f"bc_h{h}_q{qc}")
                        nc.sync.dma_start(
                            out=bc,
                            in_=sslot.unsqueeze(0).to_broadcast([65, 512]))
                    else:
                        bc = fin.tile([65, 512], f32, tag="bc", bufs=4,
                                      name=f"bc_h{h}_q{qc}")
                        if tailmode == "nobc":
                            nc.vector.tensor_copy(bc[0:1, :], rc1)
                        else:
                            nc.gpsimd.partition_broadcast(bc, rc1)
                    tl[("bc", qc)] = bc
                else:
                    if tailmode == "evonly":
                        return
                    eng = nc.gpsimd if tailmode == "gmul" else nc.vector
                    eng.tensor_mul(ost[:, sl], tl[("ev", qc)],
                                   tl[("bc", qc)])
                    if tailmode == "nodma":
                        return
                    dma = nc.sync.dma_start if outsync else nc.gpsimd.dma_start
                    dma(out=out_d[h][:, sl], in_=ost[1:65, sl])

            def emit_av(h, g):
                ots, _, _ = state[h]
                p, qc = divmod(g, 4)
                et = state[(h, "et")][g % ebufs]
                vp_v = state[(h, "vp")]
                nc.tensor.matmul(ots[qc], vp_v[:, p, :65], et[:, 0:512],
                                 start=(p == 0), stop=False)
                nc.tensor.matmul(ots[qc], vp_v[:, p + NP, :65],
                                 et[:, 512:1024],
                                 start=False, stop=(p == NP - 1))

            with loop_ctx:
                NGG = HPC * NG

                def head_setup(h):
                    qt_sb = io.tile([128, S], bf16, tag="qt")
                    nc.sync.dma_start(out=qt_sb[:, :1024],
                                      in_=qt_d[h][:, :1024])
                    kt_sb = io.tile([128, NP * 128], bf16, tag="kt")
                    nc.sync.dma_start(out=kt_sb, in_=kt_d[h])
                    nc.sync.dma_start(out=qt_sb[:, 1024:],
                                      in_=qt_d[h][:, 1024:])
                    vp_sb = io.tile([128, KT * VPW], bf16, tag="vp")
                    nc.sync.dma_start(out=vp_sb, in_=vp_d[h])
                    if h == 0:
                        for i in range(8):
                            nc.sync.dma_start(
                                out=mask_sb[:, i * 4096:(i + 1) * 4096],
                                in_=mk_d[:, i * 4096:(i + 1) * 4096])
                    ots = []
                    for qc in range(4):
                        ot = ps_o.tile([65, 512], f32, tag="ot",
                                       name=f"ot_h{h}_q{qc}")
                        ots.append(ot)
                    ost = io.tile([65, S], bf16, tag="ost", name=f"ost_h{h}")
                    state[h] = (ots, ost, {})
                    state[(h, "et")] = [None] * ebufs
                    state[(h, "qt_sb")] = qt_sb
                    state[(h, "kt_sb")] = kt_sb
                    state[(h, "vp")] = vp_sb.rearrange("p (k c) -> p k c",
                                                       k=KT)

                for gg in range(NGG):
                    h, g = divmod(gg, NG)
                    if g == 0:
                        head_setup(h)
                    p, qc = divmod(g, 4)
                    st2 = ps_s.tile([128, 1024], f32, tag="st",
                                    name=f"st_h{h}_g{g}")
                    if "qk" in stages:
                        nc.tensor.matmul(
                            st2[:, 0:512],
                            state[(h, "kt_sb")][0:64, p * 128:(p + 1) * 128],
                            state[(h, "qt_sb")][0:64, qc * 512:(qc + 1) * 512],
                            start=True, stop=True, tile_position=(0, 0))
                        nc.tensor.matmul(
                            st2[:, 512:1024],
                            state[(h, "kt_sb")][64:128, p * 128:(p + 1) * 128],
                            state[(h, "qt_sb")][64:128,
                                                qc * 512:(qc + 1) * 512],
                            start=True, stop=True, tile_position=(64, 0))
                    et2 = epool.tile([128, 1024], bf16, tag="et",
                                     name=f"et_h{h}_g{g}")
                    state[(h, "et")][g % ebufs] = et2
                    if "exp" in stages:
                        nc.scalar.activation(et2, st2, Exp,
                                             scale=1.0 / np.sqrt(DK))
                    if "mask" in stages:
                        nc.vector.tensor_mul(et2, et2, mask_v[:, g, :])
                    if "av" in stages and gg >= avlag:
                        g2 = gg - avlag
                        emit_av(g2 // NG, g2 % NG)
                    if h > 0 and "tail" in stages:
                        if tailsched is not None:
                            if g in tailsched:
                                emit_tail_piece(h - 1, tailsched.index(g))
                        elif (g % tailspread == 0
                              and g // tailspread < NTAIL):
                            emit_tail_piece(h - 1, g // tailspread)

                # epilogue: flush last AVs interleaved with last head's tail
                ep = []
                if "av" in stages:
                    for gg in range(NGG - avlag, NGG):
                        ep.append(("av", gg))
                if "tail" in stages:
                    for step in range(NTAIL):
                        ep.append(("tail", step))
                # interleave: av, tail, av, tail ...
                avs = [e for e in ep if e[0] == "av"]
                tls = [e for e in ep if e[0] == "tail"]
                inter = []
                while avs or tls:
                    if avs:
                        inter.append(avs.pop(0))
                    if tls:
                        inter.append(tls.pop(0))
                for kind, val in inter:
                    if kind == "av":
                        emit_av(val // NG, val % NG)
                    else:
                        emit_tail_piece(HPC - 1, val)

    if compile:
        nc.compile()
    return nc


def _get_nc():
    if "nc" not in _CACHE:
        _CACHE["nc"] = _build()
    return _CACHE["nc"]


def _shard(Q, K, V, attn_mask):
    """Host-side marshalling: shard/transposes per core."""
    Q = np.asarray(Q, np.float32)
    K = np.asarray(K, np.float32)
    V = np.asarray(V, np.float32)
    attn_mask = np.asarray(attn_mask, bool)

    # keep2[b][128, g=(p,qc), 1024] = [keepT(tile p) | keepT(tile p+8)] for
    # q columns qc*512:(qc+1)*512, where keepT[b, kp, t, q] = ~mask[b, q, k]
    keep = (~attn_mask[:, 0]).astype(BF16)                   # [B, q, k]
    mkT = keep.transpose(0, 2, 1)                            # [B, k, q]
    mkT = mkT.reshape(B, KT, 128, S)                         # [B, t, kp, q]
    # -> [B, 128, p, qc, half, 512]
    m2 = np.empty((B, 128, NP, 4, 2, 512), BF16)
    for p in range(NP):
        for qc in range(4):
            m2[:, :, p, qc, 0, :] = mkT[:, p, :, qc * 512:(qc + 1) * 512]
            m2[:, :, p, qc, 1, :] = mkT[:, p + NP, :, qc * 512:(qc + 1) * 512]
    m2 = np.ascontiguousarray(m2).reshape(B, 128, NG * 1024)

    in_maps = []
    for c in range(NCORES):
        b = c // (NCORES // B)
        h0 = (c % (NCORES // B)) * HPC
        QT = np.ascontiguousarray(
            Q[b, h0:h0 + HPC].transpose(0, 2, 1)).astype(BF16)  # [HPC, DK, S]
        QT2 = np.concatenate([QT, QT], axis=1)                  # [HPC, 128, S]
        KTt = np.ascontiguousarray(
            K[b, h0:h0 + HPC].transpose(0, 2, 1)).astype(BF16)  # [HPC, DK, S]
        # kt[h, 0:64, p*128:...] = tile p; kt[h, 64:128, ...] = tile p+8
        kt2 = np.empty((HPC, 128, NP * 128), BF16)
        kt2[:, 0:64, :] = KTt[:, :, :NP * 128]
        kt2[:, 64:128, :] = KTt[:, :, NP * 128:]
        vp = np.zeros((HPC, 128, KT, VPW), BF16)
        vp[:, :, :, 0] = 1.0
        vp[:, :, :, 1:DK + 1] = V[b, h0:h0 + HPC].astype(BF16).reshape(
            HPC, KT, 128, DK).transpose(0, 2, 1, 3)
        in_maps.append({
            "qt": QT2,
            "kt": kt2,
            "vp": np.ascontiguousarray(vp).reshape(HPC, 128, KT * VPW),
            "maskt": m2[b],
        })
    return in_maps


def kernel(Q, K, V, attn_mask):
    from concourse.bass_utils import run_bass_kernel_spmd

    nc = _get_nc()
    in_maps = _shard(Q, K, V, attn_mask)
    res = run_bass_kernel_spmd(nc, in_maps, list(range(NCORES)))
    out = np.empty((B, H, S, DK), np.float32)
    for c in range(NCORES):
        b = c // (NCORES // B)
        h0 = (c % (NCORES // B)) * HPC
        out[b, h0:h0 + HPC] = res.results[c]["out"].astype(
            np.float32).transpose(0, 2, 1)
    return out
